# revision 26
# baseline (speedup 1.0000x reference)
"""MoE feed-forward (top-2 of 8 experts, SwiGLU) on 8 Trainium2 NeuronCores.

Strategy (expert parallel):
  - Router (logits/softmax/top-k) computed on host with jax-CPU eager ops,
    mirroring the reference op-for-op so the routing decisions and the
    router_probs / topk_idx outputs match a CPU-run reference bit-exactly.
  - Tokens are gathered per expert on host; core e processes expert e's
    tokens (padded to a common capacity C so the 8 cores run one SPMD
    program).
  - Each core computes Y_e = (silu(X@wg+bg) * (X@wu+bu)) @ wd for its
    gathered tokens.  Matmuls run in fp16 (full TensorE rate; PSUM
    accumulation is fp32 throughout; end-to-end ~5e-4 rel err).  Setting
    MOE_GU_DT=f32r switches the gate/up matmuls to float32r (rounded-
    mantissa fp32, ~3.4e-4) at ~4% more time.
  - Host applies the top-k combine weights and the (weighted) down bias
    and scatters back to token order, accumulating experts in the same
    order as the reference scan.
"""

import os

import numpy as np

D = 1024          # d_model
F = 4096          # d_ff
E = 8             # experts
TOPK = 2
NCORES = 8
KD = D // 128     # 8 contraction tiles for gate/up
MF = F // 128     # 32 f-tiles

_NC_CACHE = {}
LAST_RESULTS = None  # BassKernelResults of the most recent device run
GU_DT = os.environ.get("MOE_GU_DT", "f16")  # gate/up matmul dtype ("f32r"
# is ~1.5x more exact but ~4% slower; both are well under 1e-3 rel err)


def _gu_subtiles(C):
    """Decompose C into token sub-tile widths, each <=512 (one PSUM bank of
    fp32) and >=256 where possible (float32r runs 4x slower below 256)."""
    n512, rem = divmod(C, 512)
    widths = [512] * n512
    if rem and rem < 256 and n512 >= 1:
        # borrow from a 512 so both parts are >=256
        widths = [512] * (n512 - 1) + [256 + rem, 256]
    elif rem:
        widths.append(rem)
    subs = []
    c0 = 0
    for w in sorted(widths):  # smallest first: first matmul waits less DMA
        subs.append((c0, w))
        c0 += w
    assert c0 == C
    return subs


def _token_blocks(C):
    """128-wide token blocks (the last may be 64) for the down matmul."""
    blocks = []
    t0 = 0
    while t0 < C:
        w = min(128, C - t0)
        blocks.append((t0, w))
        t0 += w
    return blocks


def _build_nc(C, gu_dt="f32r"):
    """Build + compile the SPMD single-core program for capacity C."""
    key = (C, gu_dt)
    if key in _NC_CACHE:
        return _NC_CACHE[key]
    import concourse.bacc as bacc
    import concourse.tile as tile
    import concourse.mybir as mybir
    from concourse.alu_op_type import AluOpType

    f32 = mybir.dt.float32
    f32r = mybir.dt.float32r
    f16 = mybir.dt.float16
    subs = _gu_subtiles(C)

    gdt = f32r if gu_dt == "f32r" else f16
    nc = bacc.Bacc("TRN2")
    xt_d = nc.dram_tensor("xt", (D, C), gdt, kind="ExternalInput").ap()
    wg_d = nc.dram_tensor("wg", (D, F), gdt, kind="ExternalInput").ap()
    wu_d = nc.dram_tensor("wu", (D, F), gdt, kind="ExternalInput").ap()
    wd_d = nc.dram_tensor("wd", (F, D), f16, kind="ExternalInput").ap()
    bg_d = nc.dram_tensor("bg", (MF, 128), f32, kind="ExternalInput").ap()
    bu_d = nc.dram_tensor("bu", (MF, 128), f32, kind="ExternalInput").ap()
    y_d = nc.dram_tensor("y", (C, D), f32, kind="ExternalOutput").ap()

    with tile.TileContext(nc) as tc:
        with tc.tile_pool(name="big", bufs=1) as big, \
             tc.tile_pool(name="wgu", bufs=3) as wgu, \
             tc.tile_pool(name="wdp", bufs=5) as wdp, \
             tc.tile_pool(name="sp", bufs=3) as sp, \
             tc.tile_pool(name="yp", bufs=4) as yp, \
             tc.tile_pool(name="pgu", bufs=2, space="PSUM") as pgu, \
             tc.tile_pool(name="pyp", bufs=6, space="PSUM") as pyp:

            # Weights for the first f-tile go on the DMA ring before X^T so
            # the first matmul can start as early as possible.
            def load_wgu(mf):
                wg_t = wgu.tile([128, KD, 128], gdt, tag="wg",
                                name=f"wg_t{mf}")
                nc.sync.dma_start(
                    wg_t, wg_d[:, mf * 128:(mf + 1) * 128]
                    .rearrange("(k p) f -> p k f", p=128))
                wu_t = wgu.tile([128, KD, 128], gdt, tag="wu",
                                name=f"wu_t{mf}")
                nc.sync.dma_start(
                    wu_t, wu_d[:, mf * 128:(mf + 1) * 128]
                    .rearrange("(k p) f -> p k f", p=128))
                return wg_t, wu_t

            bg_sb = big.tile([128, MF], f32)
            nc.sync.dma_start(bg_sb, bg_d.rearrange("m p -> p m"))
            bu_sb = big.tile([128, MF], f32)
            nc.sync.dma_start(bu_sb, bu_d.rearrange("m p -> p m"))
            wgu0 = load_wgu(0)

            # Resident tensors: X^T and hidden H.  X^T arrives in
            # sub-tile-major order so the first G/U sub-tile is ready first.
            # X^T rides the scalar engine's DMA ring (idle until phase B)
            # so the sync ring stays exclusive to the weight stream.
            xt_sb = big.tile([128, KD, C], gdt)
            for (c0, w) in subs:
                for k in range(KD):
                    nc.scalar.dma_start(
                        xt_sb[:, k, c0:c0 + w],
                        xt_d[k * 128:(k + 1) * 128, c0:c0 + w])
            h_sb = big.tile([128, MF, C], f16)
            # (Keeping wd resident in SBUF was tried and measured SLOWER:
            # the 8.4 MB bulk load interferes with phase-A weight streaming
            # on either DMA ring.  Streaming wd per group wins.)
            wd_res = False
            if wd_res:
                wd_sb = big.tile([128, MF, D], f16)
                for kf2 in range(MF // 2):
                    nc.scalar.dma_start(
                        wd_sb[:, kf2 * 2:kf2 * 2 + 2, :],
                        wd_d[kf2 * 256:(kf2 + 1) * 256, :]
                        .rearrange("(two p) d -> p two d", p=128))

            # Phase A: H[:, mf, :] = silu(X@wg + bg) * (X@wu + bu), f-tile at
            # a time.  Weights stream once; X^T stays resident.
            for mf in range(MF):
                wg_t, wu_t = wgu0 if mf == 0 else load_wgu(mf)
                for (c0, w) in subs:
                    pg = pgu.tile([128, w], f32, tag="p")
                    for k in range(KD):
                        nc.tensor.matmul(pg, wg_t[:, k, :],
                                         xt_sb[:, k, c0:c0 + w],
                                         start=(k == 0), stop=(k == KD - 1))
                    pu = pgu.tile([128, w], f32, tag="p")
                    for k in range(KD):
                        nc.tensor.matmul(pu, wu_t[:, k, :],
                                         xt_sb[:, k, c0:c0 + w],
                                         start=(k == 0), stop=(k == KD - 1))
                    # silu(G+bg) * (U+bu), with silu(v) = v * sigmoid(v):
                    s = sp.tile([128, w], f32, tag="s")
                    nc.scalar.activation(
                        s, pg, mybir.ActivationFunctionType.Sigmoid,
                        bias=bg_sb[:, mf:mf + 1])
                    t = sp.tile([128, w], f32, tag="t")
                    nc.vector.scalar_tensor_tensor(
                        t, pg, bg_sb[:, mf:mf + 1], s,
                        op0=AluOpType.add, op1=AluOpType.mult)
                    # H = (pu + bu) * t, written as fp16
                    nc.vector.scalar_tensor_tensor(
                        h_sb[:, mf, c0:c0 + w], pu, bu_sb[:, mf:mf + 1], t,
                        op0=AluOpType.add, op1=AluOpType.mult)

            # Phase B: Y[t, :] = H^T[t, :] @ wd, token-block groups of <=3
            # (3 blocks x 2 d-halves = 6 PSUM banks), wd streamed per group
            # in 2-f-tile batched DMAs.  Output copies go out on the scalar
            # engine's DMA ring to keep the sync ring free for weights.
            tbs = _token_blocks(C)
            groups = [tbs[g:g + 3] for g in range(0, len(tbs), 3)]
            for gi, grp in enumerate(groups):
                pys = [[pyp.tile([128, 512], f32, tag="py",
                                 name=f"py_{gi}_{tb}_{md}")
                        for md in range(2)] for tb in range(len(grp))]
                for kf2 in range(MF // 2):
                    if wd_res:
                        wd_t = wd_sb[:, kf2 * 2:kf2 * 2 + 2, :]
                    else:
                        wd_t = wdp.tile([128, 2, D], f16, tag="wd")
                        nc.sync.dma_start(
                            wd_t, wd_d[kf2 * 256:(kf2 + 1) * 256, :]
                            .rearrange("(two p) d -> p two d", p=128))
                    for i in range(2):
                        kf = kf2 * 2 + i
                        for tb, (t0, tw) in enumerate(grp):
                            for md in range(2):
                                nc.tensor.matmul(
                                    pys[tb][md][:tw, :],
                                    h_sb[:, kf, t0:t0 + tw],
                                    wd_t[:, i, md * 512:(md + 1) * 512],
                                    start=(kf == 0), stop=(kf == MF - 1))
                for tb, (t0, tw) in enumerate(grp):
                    for md in range(2):
                        # alternate copy engine (ACT/DVE) and DMA ring so the
                        # PSUM drain at group boundaries isn't serialized
                        yt = yp.tile([128, 512], f32, tag="y")
                        if md == 0:
                            nc.scalar.copy(yt[:tw, :], pys[tb][md][:tw, :])
                            nc.scalar.dma_start(
                                y_d[t0:t0 + tw, md * 512:(md + 1) * 512],
                                yt[:tw, :])
                        else:
                            nc.vector.tensor_copy(yt[:tw, :],
                                                  pys[tb][md][:tw, :])
                            nc.sync.dma_start(
                                y_d[t0:t0 + tw, md * 512:(md + 1) * 512],
                                yt[:tw, :])

    nc.compile()
    _NC_CACHE[key] = nc
    return nc


def _router_cpu(flat, router_w, router_b):
    """Mirror the reference router eagerly on jax-CPU (op-for-op identical
    numerics to a CPU-run reference)."""
    import jax
    import jax.numpy as jnp
    from jax import lax

    cpu = jax.devices("cpu")[0]
    fl = jax.device_put(flat, cpu)
    rw = jax.device_put(router_w, cpu)
    rb = jax.device_put(router_b, cpu)
    logits = fl @ rw + rb
    probs = jax.nn.softmax(logits, axis=-1)
    topk_vals, topk_idx = lax.top_k(logits, TOPK)
    topk_probs = jax.nn.softmax(topk_vals, axis=-1)
    return (np.asarray(probs), np.asarray(topk_idx), np.asarray(topk_probs))


def kernel(x, router_w, router_b, wg, bg, wu, bu, wd, bd):
    global LAST_RESULTS
    from concourse import bass_utils

    x = np.asarray(x, dtype=np.float32)
    router_w = np.asarray(router_w, dtype=np.float32)
    router_b = np.asarray(router_b, dtype=np.float32)
    wg = np.asarray(wg, dtype=np.float32)
    bg = np.asarray(bg, dtype=np.float32)
    wu = np.asarray(wu, dtype=np.float32)
    bu = np.asarray(bu, dtype=np.float32)
    wd = np.asarray(wd, dtype=np.float32)
    bd = np.asarray(bd, dtype=np.float32)

    B, S, _ = x.shape
    T = B * S
    flat = np.ascontiguousarray(x.reshape(T, D))

    probs, ti, tp = _router_cpu(flat, router_w, router_b)

    # Per-expert token rows and combine weights.
    rows_e, w_e = [], []
    for e in range(E):
        sel = ti == e                      # [T, 2]; at most one hit per row
        rows = np.nonzero(sel.any(axis=1))[0]
        w = np.where(sel[rows, 0], tp[rows, 0], tp[rows, 1])
        rows_e.append(rows)
        w_e.append(w.astype(np.float32))

    max_load = max(len(r) for r in rows_e)
    C = max(512, -(-max_load // 64) * 64)
    nc = _build_nc(C, GU_DT)

    gu_np = np.float32 if GU_DT == "f32r" else np.float16
    in_maps = []
    for e in range(E):
        xp = np.zeros((C, D), np.float32)
        xp[: len(rows_e[e])] = flat[rows_e[e]]
        in_maps.append({
            "xt": np.ascontiguousarray(xp.T.astype(gu_np)),
            "wg": np.ascontiguousarray(wg[e].astype(gu_np)),
            "wu": np.ascontiguousarray(wu[e].astype(gu_np)),
            "wd": np.ascontiguousarray(wd[e].astype(np.float16)),
            "bg": np.ascontiguousarray(bg[e].reshape(MF, 128)),
            "bu": np.ascontiguousarray(bu[e].reshape(MF, 128)),
        })

    res = bass_utils.run_bass_kernel_spmd(nc, in_maps, core_ids=list(range(NCORES)))
    LAST_RESULTS = res

    out_flat = np.zeros((T, D), np.float32)
    for e in range(E):
        rows = rows_e[e]
        ye = res.results[e]["y"][: len(rows)]
        out_flat[rows] += w_e[e][:, None] * (ye + bd[e][None, :])

    return (out_flat.reshape(B, S, D),
            probs.reshape(B, S, E).astype(np.float32),
            ti.reshape(B, S, TOPK).astype(np.int32))


# revision 27
# speedup vs baseline: 1.0009x; 1.0009x over previous
"""MoE feed-forward (top-2 of 8 experts, SwiGLU) on 8 Trainium2 NeuronCores.

Strategy (expert parallel):
  - Router (logits/softmax/top-k) computed on host with jax-CPU eager ops,
    mirroring the reference op-for-op so the routing decisions and the
    router_probs / topk_idx outputs match a CPU-run reference bit-exactly.
  - Tokens are gathered per expert on host; core e processes expert e's
    tokens (padded to a common capacity C so the 8 cores run one SPMD
    program).
  - Each core computes Y_e = (silu(X@wg+bg) * (X@wu+bu)) @ wd for its
    gathered tokens.  Matmuls run in fp16 (full TensorE rate; PSUM
    accumulation is fp32 throughout; end-to-end ~5e-4 rel err).  Setting
    MOE_GU_DT=f32r switches the gate/up matmuls to float32r (rounded-
    mantissa fp32, ~3.4e-4) at ~4% more time.
  - Host applies the top-k combine weights and the (weighted) down bias
    and scatters back to token order, accumulating experts in the same
    order as the reference scan.
"""

import os

import numpy as np

D = 1024          # d_model
F = 4096          # d_ff
E = 8             # experts
TOPK = 2
NCORES = 8
KD = D // 128     # 8 contraction tiles for gate/up
MF = F // 128     # 32 f-tiles

_NC_CACHE = {}
LAST_RESULTS = None  # BassKernelResults of the most recent device run
GU_DT = os.environ.get("MOE_GU_DT", "f16")  # gate/up matmul dtype ("f32r"
# is ~1.5x more exact but ~4% slower; both are well under 1e-3 rel err)


def _gu_subtiles(C):
    """Decompose C into token sub-tile widths, each <=512 (one PSUM bank of
    fp32) and >=256 where possible (float32r runs 4x slower below 256)."""
    n512, rem = divmod(C, 512)
    widths = [512] * n512
    if rem and rem < 256 and n512 >= 1:
        # borrow from a 512 so both parts are >=256
        widths = [512] * (n512 - 1) + [256 + rem, 256]
    elif rem:
        widths.append(rem)
    subs = []
    c0 = 0
    for w in widths:
        subs.append((c0, w))
        c0 += w
    assert c0 == C
    return subs


def _token_blocks(C):
    """128-wide token blocks (the last may be 64) for the down matmul."""
    blocks = []
    t0 = 0
    while t0 < C:
        w = min(128, C - t0)
        blocks.append((t0, w))
        t0 += w
    return blocks


def _build_nc(C, gu_dt="f32r"):
    """Build + compile the SPMD single-core program for capacity C."""
    key = (C, gu_dt)
    if key in _NC_CACHE:
        return _NC_CACHE[key]
    import concourse.bacc as bacc
    import concourse.tile as tile
    import concourse.mybir as mybir
    from concourse.alu_op_type import AluOpType

    f32 = mybir.dt.float32
    f32r = mybir.dt.float32r
    f16 = mybir.dt.float16
    subs = _gu_subtiles(C)

    gdt = f32r if gu_dt == "f32r" else f16
    nc = bacc.Bacc("TRN2")
    xt_d = nc.dram_tensor("xt", (D, C), gdt, kind="ExternalInput").ap()
    wg_d = nc.dram_tensor("wg", (D, F), gdt, kind="ExternalInput").ap()
    wu_d = nc.dram_tensor("wu", (D, F), gdt, kind="ExternalInput").ap()
    wd_d = nc.dram_tensor("wd", (F, D), f16, kind="ExternalInput").ap()
    bg_d = nc.dram_tensor("bg", (MF, 128), f32, kind="ExternalInput").ap()
    bu_d = nc.dram_tensor("bu", (MF, 128), f32, kind="ExternalInput").ap()
    y_d = nc.dram_tensor("y", (C, D), f32, kind="ExternalOutput").ap()

    with tile.TileContext(nc) as tc:
        with tc.tile_pool(name="big", bufs=1) as big, \
             tc.tile_pool(name="wgu", bufs=2) as wgu, \
             tc.tile_pool(name="wdp", bufs=5) as wdp, \
             tc.tile_pool(name="sp", bufs=3) as sp, \
             tc.tile_pool(name="yp", bufs=4) as yp, \
             tc.tile_pool(name="pgu", bufs=2, space="PSUM") as pgu, \
             tc.tile_pool(name="pyp", bufs=6, space="PSUM") as pyp:

            # Weights for the first f-tile go on the DMA ring before X^T so
            # the first matmul can start as early as possible.
            def load_wgu(mf):
                wg_t = wgu.tile([128, KD, 128], gdt, tag="wg",
                                name=f"wg_t{mf}")
                nc.sync.dma_start(
                    wg_t, wg_d[:, mf * 128:(mf + 1) * 128]
                    .rearrange("(k p) f -> p k f", p=128))
                wu_t = wgu.tile([128, KD, 128], gdt, tag="wu",
                                name=f"wu_t{mf}")
                nc.sync.dma_start(
                    wu_t, wu_d[:, mf * 128:(mf + 1) * 128]
                    .rearrange("(k p) f -> p k f", p=128))
                return wg_t, wu_t

            bg_sb = big.tile([128, MF], f32)
            nc.sync.dma_start(bg_sb, bg_d.rearrange("m p -> p m"))
            bu_sb = big.tile([128, MF], f32)
            nc.sync.dma_start(bu_sb, bu_d.rearrange("m p -> p m"))
            wgu0 = load_wgu(0)

            # Resident tensors: X^T and hidden H.  X^T arrives in
            # sub-tile-major order so the first G/U sub-tile is ready first.
            xt_sb = big.tile([128, KD, C], gdt)
            for (c0, w) in subs:
                for k in range(KD):
                    nc.sync.dma_start(
                        xt_sb[:, k, c0:c0 + w],
                        xt_d[k * 128:(k + 1) * 128, c0:c0 + w])
            h_sb = big.tile([128, MF, C], f16)
            # (Keeping wd resident in SBUF was tried and measured SLOWER:
            # the 8.4 MB bulk load interferes with phase-A weight streaming
            # on either DMA ring.  Streaming wd per group wins.)
            wd_res = False
            if wd_res:
                wd_sb = big.tile([128, MF, D], f16)
                for kf2 in range(MF // 2):
                    nc.scalar.dma_start(
                        wd_sb[:, kf2 * 2:kf2 * 2 + 2, :],
                        wd_d[kf2 * 256:(kf2 + 1) * 256, :]
                        .rearrange("(two p) d -> p two d", p=128))

            # Phase A: H[:, mf, :] = silu(X@wg + bg) * (X@wu + bu), f-tile at
            # a time.  Weights stream once; X^T stays resident.
            for mf in range(MF):
                wg_t, wu_t = wgu0 if mf == 0 else load_wgu(mf)
                for (c0, w) in subs:
                    pg = pgu.tile([128, w], f32, tag="p")
                    for k in range(KD):
                        nc.tensor.matmul(pg, wg_t[:, k, :],
                                         xt_sb[:, k, c0:c0 + w],
                                         start=(k == 0), stop=(k == KD - 1))
                    pu = pgu.tile([128, w], f32, tag="p")
                    for k in range(KD):
                        nc.tensor.matmul(pu, wu_t[:, k, :],
                                         xt_sb[:, k, c0:c0 + w],
                                         start=(k == 0), stop=(k == KD - 1))
                    # silu(G+bg) * (U+bu), with silu(v) = v * sigmoid(v):
                    s = sp.tile([128, w], f32, tag="s")
                    nc.scalar.activation(
                        s, pg, mybir.ActivationFunctionType.Sigmoid,
                        bias=bg_sb[:, mf:mf + 1])
                    t = sp.tile([128, w], f32, tag="t")
                    nc.vector.scalar_tensor_tensor(
                        t, pg, bg_sb[:, mf:mf + 1], s,
                        op0=AluOpType.add, op1=AluOpType.mult)
                    # H = (pu + bu) * t, written as fp16
                    nc.vector.scalar_tensor_tensor(
                        h_sb[:, mf, c0:c0 + w], pu, bu_sb[:, mf:mf + 1], t,
                        op0=AluOpType.add, op1=AluOpType.mult)

            # Phase B: Y[t, :] = H^T[t, :] @ wd, token-block groups of <=3
            # (3 blocks x 2 d-halves = 6 PSUM banks), wd streamed per group
            # in 2-f-tile batched DMAs.  Output copies go out on the scalar
            # engine's DMA ring to keep the sync ring free for weights.
            tbs = _token_blocks(C)
            groups = [tbs[g:g + 3] for g in range(0, len(tbs), 3)]
            for gi, grp in enumerate(groups):
                pys = [[pyp.tile([128, 512], f32, tag="py",
                                 name=f"py_{gi}_{tb}_{md}")
                        for md in range(2)] for tb in range(len(grp))]
                for kf2 in range(MF // 2):
                    if wd_res:
                        wd_t = wd_sb[:, kf2 * 2:kf2 * 2 + 2, :]
                    else:
                        wd_t = wdp.tile([128, 2, D], f16, tag="wd")
                        nc.sync.dma_start(
                            wd_t, wd_d[kf2 * 256:(kf2 + 1) * 256, :]
                            .rearrange("(two p) d -> p two d", p=128))
                    for i in range(2):
                        kf = kf2 * 2 + i
                        for tb, (t0, tw) in enumerate(grp):
                            for md in range(2):
                                nc.tensor.matmul(
                                    pys[tb][md][:tw, :],
                                    h_sb[:, kf, t0:t0 + tw],
                                    wd_t[:, i, md * 512:(md + 1) * 512],
                                    start=(kf == 0), stop=(kf == MF - 1))
                for tb, (t0, tw) in enumerate(grp):
                    for md in range(2):
                        yt = yp.tile([128, 512], f32, tag="y")
                        nc.scalar.copy(yt[:tw, :], pys[tb][md][:tw, :])
                        nc.scalar.dma_start(
                            y_d[t0:t0 + tw, md * 512:(md + 1) * 512],
                            yt[:tw, :])

    nc.compile()
    _NC_CACHE[key] = nc
    return nc


def _router_cpu(flat, router_w, router_b):
    """Mirror the reference router eagerly on jax-CPU (op-for-op identical
    numerics to a CPU-run reference)."""
    import jax
    import jax.numpy as jnp
    from jax import lax

    cpu = jax.devices("cpu")[0]
    fl = jax.device_put(flat, cpu)
    rw = jax.device_put(router_w, cpu)
    rb = jax.device_put(router_b, cpu)
    logits = fl @ rw + rb
    probs = jax.nn.softmax(logits, axis=-1)
    topk_vals, topk_idx = lax.top_k(logits, TOPK)
    topk_probs = jax.nn.softmax(topk_vals, axis=-1)
    return (np.asarray(probs), np.asarray(topk_idx), np.asarray(topk_probs))


def kernel(x, router_w, router_b, wg, bg, wu, bu, wd, bd):
    global LAST_RESULTS
    from concourse import bass_utils

    x = np.asarray(x, dtype=np.float32)
    router_w = np.asarray(router_w, dtype=np.float32)
    router_b = np.asarray(router_b, dtype=np.float32)
    wg = np.asarray(wg, dtype=np.float32)
    bg = np.asarray(bg, dtype=np.float32)
    wu = np.asarray(wu, dtype=np.float32)
    bu = np.asarray(bu, dtype=np.float32)
    wd = np.asarray(wd, dtype=np.float32)
    bd = np.asarray(bd, dtype=np.float32)

    B, S, _ = x.shape
    T = B * S
    flat = np.ascontiguousarray(x.reshape(T, D))

    probs, ti, tp = _router_cpu(flat, router_w, router_b)

    # Per-expert token rows and combine weights.
    rows_e, w_e = [], []
    for e in range(E):
        sel = ti == e                      # [T, 2]; at most one hit per row
        rows = np.nonzero(sel.any(axis=1))[0]
        w = np.where(sel[rows, 0], tp[rows, 0], tp[rows, 1])
        rows_e.append(rows)
        w_e.append(w.astype(np.float32))

    max_load = max(len(r) for r in rows_e)
    C = max(512, -(-max_load // 64) * 64)
    nc = _build_nc(C, GU_DT)

    gu_np = np.float32 if GU_DT == "f32r" else np.float16
    in_maps = []
    for e in range(E):
        xp = np.zeros((C, D), np.float32)
        xp[: len(rows_e[e])] = flat[rows_e[e]]
        in_maps.append({
            "xt": np.ascontiguousarray(xp.T.astype(gu_np)),
            "wg": np.ascontiguousarray(wg[e].astype(gu_np)),
            "wu": np.ascontiguousarray(wu[e].astype(gu_np)),
            "wd": np.ascontiguousarray(wd[e].astype(np.float16)),
            "bg": np.ascontiguousarray(bg[e].reshape(MF, 128)),
            "bu": np.ascontiguousarray(bu[e].reshape(MF, 128)),
        })

    res = bass_utils.run_bass_kernel_spmd(nc, in_maps, core_ids=list(range(NCORES)))
    LAST_RESULTS = res

    out_flat = np.zeros((T, D), np.float32)
    for e in range(E):
        rows = rows_e[e]
        ye = res.results[e]["y"][: len(rows)]
        out_flat[rows] += w_e[e][:, None] * (ye + bd[e][None, :])

    return (out_flat.reshape(B, S, D),
            probs.reshape(B, S, E).astype(np.float32),
            ti.reshape(B, S, TOPK).astype(np.int32))


# revision 28
# speedup vs baseline: 1.0125x; 1.0116x over previous
"""MoE feed-forward (top-2 of 8 experts, SwiGLU) on 8 Trainium2 NeuronCores.

Strategy (expert parallel):
  - Router (logits/softmax/top-k) computed on host with jax-CPU eager ops,
    mirroring the reference op-for-op so the routing decisions and the
    router_probs / topk_idx outputs match a CPU-run reference bit-exactly.
  - Tokens are gathered per expert on host; core e processes expert e's
    tokens (padded to a common capacity C so the 8 cores run one SPMD
    program).
  - Each core computes Y_e = (silu(X@wg+bg) * (X@wu+bu)) @ wd for its
    gathered tokens.  Matmuls run in fp16 (full TensorE rate; PSUM
    accumulation is fp32 throughout; end-to-end ~5e-4 rel err).  Setting
    MOE_GU_DT=f32r switches the gate/up matmuls to float32r (rounded-
    mantissa fp32, ~3.4e-4) at ~4% more time.
  - Host applies the top-k combine weights and the (weighted) down bias
    and scatters back to token order, accumulating experts in the same
    order as the reference scan.
"""

import os

import numpy as np

D = 1024          # d_model
F = 4096          # d_ff
E = 8             # experts
TOPK = 2
NCORES = 8
KD = D // 128     # 8 contraction tiles for gate/up
MF = F // 128     # 32 f-tiles

_NC_CACHE = {}
LAST_RESULTS = None  # BassKernelResults of the most recent device run
GU_DT = os.environ.get("MOE_GU_DT", "f16")  # gate/up matmul dtype ("f32r"
# is ~1.5x more exact but ~4% slower; both are well under 1e-3 rel err)


def _gu_subtiles(C):
    """Decompose C into token sub-tile widths, each <=512 (one PSUM bank of
    fp32) and >=256 where possible (float32r runs 4x slower below 256)."""
    n512, rem = divmod(C, 512)
    widths = [512] * n512
    if rem and rem < 256 and n512 >= 1:
        # borrow from a 512 so both parts are >=256
        widths = [512] * (n512 - 1) + [256 + rem, 256]
    elif rem:
        widths.append(rem)
    subs = []
    c0 = 0
    for w in widths:
        subs.append((c0, w))
        c0 += w
    assert c0 == C
    return subs


def _token_blocks(C):
    """128-wide token blocks (the last may be 64) for the down matmul."""
    blocks = []
    t0 = 0
    while t0 < C:
        w = min(128, C - t0)
        blocks.append((t0, w))
        t0 += w
    return blocks


def _build_nc(C, gu_dt="f32r"):
    """Build + compile the SPMD single-core program for capacity C."""
    key = (C, gu_dt)
    if key in _NC_CACHE:
        return _NC_CACHE[key]
    import concourse.bacc as bacc
    import concourse.tile as tile
    import concourse.mybir as mybir
    from concourse.alu_op_type import AluOpType

    f32 = mybir.dt.float32
    f32r = mybir.dt.float32r
    f16 = mybir.dt.float16
    subs = _gu_subtiles(C)

    gdt = f32r if gu_dt == "f32r" else f16
    nc = bacc.Bacc("TRN2")
    xt_d = nc.dram_tensor("xt", (D, C), gdt, kind="ExternalInput").ap()
    wg_d = nc.dram_tensor("wg", (D, F), gdt, kind="ExternalInput").ap()
    wu_d = nc.dram_tensor("wu", (D, F), gdt, kind="ExternalInput").ap()
    wd_d = nc.dram_tensor("wd", (F, D), f16, kind="ExternalInput").ap()
    bg_d = nc.dram_tensor("bg", (MF, 128), f32, kind="ExternalInput").ap()
    bu_d = nc.dram_tensor("bu", (MF, 128), f32, kind="ExternalInput").ap()
    y_d = nc.dram_tensor("y", (C, D), f32, kind="ExternalOutput").ap()

    with tile.TileContext(nc) as tc:
        with tc.tile_pool(name="big", bufs=1) as big, \
             tc.tile_pool(name="wgu", bufs=2) as wgu, \
             tc.tile_pool(name="wdp", bufs=5) as wdp, \
             tc.tile_pool(name="sp", bufs=3) as sp, \
             tc.tile_pool(name="yp", bufs=4) as yp, \
             tc.tile_pool(name="pgu", bufs=2, space="PSUM") as pgu, \
             tc.tile_pool(name="pyp", bufs=6, space="PSUM") as pyp:

            # Weights for the first f-tile go on the DMA ring before X^T so
            # the first matmul can start as early as possible.
            def load_wgu(mf):
                wg_t = wgu.tile([128, KD, 128], gdt, tag="wg",
                                name=f"wg_t{mf}")
                nc.sync.dma_start(
                    wg_t, wg_d[:, mf * 128:(mf + 1) * 128]
                    .rearrange("(k p) f -> p k f", p=128))
                wu_t = wgu.tile([128, KD, 128], gdt, tag="wu",
                                name=f"wu_t{mf}")
                nc.sync.dma_start(
                    wu_t, wu_d[:, mf * 128:(mf + 1) * 128]
                    .rearrange("(k p) f -> p k f", p=128))
                return wg_t, wu_t

            bg_sb = big.tile([128, MF], f32)
            nc.sync.dma_start(bg_sb, bg_d.rearrange("m p -> p m"))
            bu_sb = big.tile([128, MF], f32)
            nc.sync.dma_start(bu_sb, bu_d.rearrange("m p -> p m"))
            wgu0 = load_wgu(0)

            # Resident tensors: X^T and hidden H.  X^T arrives in
            # sub-tile-major order so the first G/U sub-tile is ready first.
            # One 3D-AP DMA per sub-tile (a single DMA already fans out
            # across all 16 SDMA engines; fewer DMAs = less issue latency
            # before the first matmul can start).
            xt_sb = big.tile([128, KD, C], gdt)
            for (c0, w) in subs:
                nc.sync.dma_start(
                    xt_sb[:, :, c0:c0 + w],
                    xt_d[:, c0:c0 + w].rearrange("(k p) c -> p k c", p=128))
            h_sb = big.tile([128, MF, C], f16)
            # (Keeping wd resident in SBUF was tried and measured SLOWER:
            # the 8.4 MB bulk load interferes with phase-A weight streaming
            # on either DMA ring.  Streaming wd per group wins.)
            wd_res = False
            if wd_res:
                wd_sb = big.tile([128, MF, D], f16)
                for kf2 in range(MF // 2):
                    nc.scalar.dma_start(
                        wd_sb[:, kf2 * 2:kf2 * 2 + 2, :],
                        wd_d[kf2 * 256:(kf2 + 1) * 256, :]
                        .rearrange("(two p) d -> p two d", p=128))

            # Phase A: H[:, mf, :] = silu(X@wg + bg) * (X@wu + bu), f-tile at
            # a time.  Weights stream once; X^T stays resident.
            for mf in range(MF):
                wg_t, wu_t = wgu0 if mf == 0 else load_wgu(mf)
                for (c0, w) in subs:
                    pg = pgu.tile([128, w], f32, tag="p")
                    for k in range(KD):
                        nc.tensor.matmul(pg, wg_t[:, k, :],
                                         xt_sb[:, k, c0:c0 + w],
                                         start=(k == 0), stop=(k == KD - 1))
                    pu = pgu.tile([128, w], f32, tag="p")
                    for k in range(KD):
                        nc.tensor.matmul(pu, wu_t[:, k, :],
                                         xt_sb[:, k, c0:c0 + w],
                                         start=(k == 0), stop=(k == KD - 1))
                    # silu(G+bg) * (U+bu), with silu(v) = v * sigmoid(v):
                    s = sp.tile([128, w], f32, tag="s")
                    nc.scalar.activation(
                        s, pg, mybir.ActivationFunctionType.Sigmoid,
                        bias=bg_sb[:, mf:mf + 1])
                    t = sp.tile([128, w], f32, tag="t")
                    nc.vector.scalar_tensor_tensor(
                        t, pg, bg_sb[:, mf:mf + 1], s,
                        op0=AluOpType.add, op1=AluOpType.mult)
                    # H = (pu + bu) * t, written as fp16
                    nc.vector.scalar_tensor_tensor(
                        h_sb[:, mf, c0:c0 + w], pu, bu_sb[:, mf:mf + 1], t,
                        op0=AluOpType.add, op1=AluOpType.mult)

            # Phase B: Y[t, :] = H^T[t, :] @ wd, token-block groups of <=3
            # (3 blocks x 2 d-halves = 6 PSUM banks), wd streamed per group
            # in 2-f-tile batched DMAs.  Output copies go out on the scalar
            # engine's DMA ring to keep the sync ring free for weights.
            tbs = _token_blocks(C)
            groups = [tbs[g:g + 3] for g in range(0, len(tbs), 3)]
            for gi, grp in enumerate(groups):
                pys = [[pyp.tile([128, 512], f32, tag="py",
                                 name=f"py_{gi}_{tb}_{md}")
                        for md in range(2)] for tb in range(len(grp))]
                for kf2 in range(MF // 2):
                    if wd_res:
                        wd_t = wd_sb[:, kf2 * 2:kf2 * 2 + 2, :]
                    else:
                        wd_t = wdp.tile([128, 2, D], f16, tag="wd")
                        nc.sync.dma_start(
                            wd_t, wd_d[kf2 * 256:(kf2 + 1) * 256, :]
                            .rearrange("(two p) d -> p two d", p=128))
                    for i in range(2):
                        kf = kf2 * 2 + i
                        for tb, (t0, tw) in enumerate(grp):
                            for md in range(2):
                                nc.tensor.matmul(
                                    pys[tb][md][:tw, :],
                                    h_sb[:, kf, t0:t0 + tw],
                                    wd_t[:, i, md * 512:(md + 1) * 512],
                                    start=(kf == 0), stop=(kf == MF - 1))
                for tb, (t0, tw) in enumerate(grp):
                    for md in range(2):
                        yt = yp.tile([128, 512], f32, tag="y")
                        nc.scalar.copy(yt[:tw, :], pys[tb][md][:tw, :])
                        nc.scalar.dma_start(
                            y_d[t0:t0 + tw, md * 512:(md + 1) * 512],
                            yt[:tw, :])

    nc.compile()
    _NC_CACHE[key] = nc
    return nc


def _router_cpu(flat, router_w, router_b):
    """Mirror the reference router eagerly on jax-CPU (op-for-op identical
    numerics to a CPU-run reference)."""
    import jax
    import jax.numpy as jnp
    from jax import lax

    cpu = jax.devices("cpu")[0]
    fl = jax.device_put(flat, cpu)
    rw = jax.device_put(router_w, cpu)
    rb = jax.device_put(router_b, cpu)
    logits = fl @ rw + rb
    probs = jax.nn.softmax(logits, axis=-1)
    topk_vals, topk_idx = lax.top_k(logits, TOPK)
    topk_probs = jax.nn.softmax(topk_vals, axis=-1)
    return (np.asarray(probs), np.asarray(topk_idx), np.asarray(topk_probs))


def kernel(x, router_w, router_b, wg, bg, wu, bu, wd, bd):
    global LAST_RESULTS
    from concourse import bass_utils

    x = np.asarray(x, dtype=np.float32)
    router_w = np.asarray(router_w, dtype=np.float32)
    router_b = np.asarray(router_b, dtype=np.float32)
    wg = np.asarray(wg, dtype=np.float32)
    bg = np.asarray(bg, dtype=np.float32)
    wu = np.asarray(wu, dtype=np.float32)
    bu = np.asarray(bu, dtype=np.float32)
    wd = np.asarray(wd, dtype=np.float32)
    bd = np.asarray(bd, dtype=np.float32)

    B, S, _ = x.shape
    T = B * S
    flat = np.ascontiguousarray(x.reshape(T, D))

    probs, ti, tp = _router_cpu(flat, router_w, router_b)

    # Per-expert token rows and combine weights.
    rows_e, w_e = [], []
    for e in range(E):
        sel = ti == e                      # [T, 2]; at most one hit per row
        rows = np.nonzero(sel.any(axis=1))[0]
        w = np.where(sel[rows, 0], tp[rows, 0], tp[rows, 1])
        rows_e.append(rows)
        w_e.append(w.astype(np.float32))

    max_load = max(len(r) for r in rows_e)
    C = max(512, -(-max_load // 64) * 64)
    nc = _build_nc(C, GU_DT)

    gu_np = np.float32 if GU_DT == "f32r" else np.float16
    in_maps = []
    for e in range(E):
        xp = np.zeros((C, D), np.float32)
        xp[: len(rows_e[e])] = flat[rows_e[e]]
        in_maps.append({
            "xt": np.ascontiguousarray(xp.T.astype(gu_np)),
            "wg": np.ascontiguousarray(wg[e].astype(gu_np)),
            "wu": np.ascontiguousarray(wu[e].astype(gu_np)),
            "wd": np.ascontiguousarray(wd[e].astype(np.float16)),
            "bg": np.ascontiguousarray(bg[e].reshape(MF, 128)),
            "bu": np.ascontiguousarray(bu[e].reshape(MF, 128)),
        })

    res = bass_utils.run_bass_kernel_spmd(nc, in_maps, core_ids=list(range(NCORES)))
    LAST_RESULTS = res

    out_flat = np.zeros((T, D), np.float32)
    for e in range(E):
        rows = rows_e[e]
        ye = res.results[e]["y"][: len(rows)]
        out_flat[rows] += w_e[e][:, None] * (ye + bd[e][None, :])

    return (out_flat.reshape(B, S, D),
            probs.reshape(B, S, E).astype(np.float32),
            ti.reshape(B, S, TOPK).astype(np.int32))
